# revision 19
# baseline (speedup 1.0000x reference)
"""Trainium2 Bass kernel for AdaptiveEmbedding T2I sims.

Reference computation (per full batch):
  cap_repr = ragged-mean(cap_embed, lens)                       (Bc, D)
  bn       = batchnorm(img_embed^T) over (Bi, R) per channel d  (Bi, D, R)
  gamma    = MLP_g(cap_repr); beta = MLP_b(cap_repr)            (Bc, D)
  out      = bn * gamma + beta                                  (Bc, Bi, D, R)
  m        = softmax(out * 10, axis=-1)
  img_vec  = l2norm(mean_r(m * out))                            (Bc, Bi, D)
  sims     = einsum('cbd,cd->bc', img_vec, l2norm(cap_repr))    (Bi, Bc)

Device algebra (what the kernel actually computes, per caption c):
  softmax weights are invariant to the +beta shift and to any per-(c,d)
  constant factor, so with A = G*gamma*rstd:
     e = exp(A * x)          x = imgT[d, (b,r)]   (raw image, d on partitions)
     S' = sum_r(e*x) / sum_r(e)
     iv = P1*S' + P2         P1 = gamma*rstd, P2 = gamma*cbn + beta
  iv = R * img_vec(un-normalized);  sims = s1 / ((sqrt(s2)+R*eps)(sqrt(s3)+eps))
  with s1 = sum_d iv*cv, s2 = sum_d iv^2, s3 = sum_d cv^2.

Sharding: captions (Bc=32) split 4-per-core across 8 cores; img + MLP params
replicated. Per-core output is its 4 sims columns; host concatenates.
"""

import sys

if "/opt/trn_rl_repo" not in sys.path:
    sys.path.insert(0, "/opt/trn_rl_repo")

import numpy as np

# Problem constants (hardcoded per spec)
Bi, R, D, Bc, T, H = 64, 36, 1024, 32, 64, 128
NCORES = 8
CL = Bc // NCORES            # captions per core = 4
BR = Bi * R                  # 2304
P = 128                      # partitions
ND = D // P                  # 8 d-chunks
GAMMA = 10.0
EPS_BN = 1e-5
EPS_L2 = 1e-8

_COMPILED = None             # cached (nc,) so repeat kernel() calls skip rebuild


def _patch_act_tables():
    """Steer the act-table chooser to `natural_log_exp_and_others` (the only
    set with both exp and ln) for every function this kernel uses, so the
    Scalar engine never swaps table sets mid-kernel (~2.7us per swap)."""
    from concourse import bacc, hw_specs, mybir

    if getattr(bacc, "_act_tables_patched", False):
        return
    orig = hw_specs.get_activation_tables
    AF = mybir.ActivationFunctionType
    mine = {AF.Exp, AF.Ln, AF.Copy, AF.Square, AF.Identity, AF.Relu}

    def patched(arch):
        tables = orig(arch)
        for name, funcs in tables.items():
            if name != "natural_log_exp_and_others":
                tables[name] = funcs - mine
        return tables

    bacc.get_activation_tables = patched
    bacc._act_tables_patched = True


def _build_graph():
    from concourse import bacc, mybir, tile
    import concourse.bass as bass

    _patch_act_tables()

    F32 = mybir.dt.float32
    BF16 = mybir.dt.bfloat16
    AF = mybir.ActivationFunctionType
    AX = mybir.AxisListType
    ALU = mybir.AluOpType

    nc = bacc.Bacc("TRN2", target_bir_lowering=False, debug=False,
                   num_devices=NCORES)

    imgT = nc.declare_dram_parameter("imgT", [D, BR], F32, isOutput=False)
    cap = nc.declare_dram_parameter("cap", [CL * T, D], F32, isOutput=False)
    wm = nc.declare_dram_parameter("wm", [CL * T, CL], F32, isOutput=False)
    Wg1 = nc.declare_dram_parameter("Wg1", [D, H], F32, isOutput=False)
    Wg2 = nc.declare_dram_parameter("Wg2", [H, D], F32, isOutput=False)
    Wb1 = nc.declare_dram_parameter("Wb1", [D, H], F32, isOutput=False)
    Wb2 = nc.declare_dram_parameter("Wb2", [H, D], F32, isOutput=False)
    bg1 = nc.declare_dram_parameter("bg1", [H, 1], F32, isOutput=False)
    bb1 = nc.declare_dram_parameter("bb1", [H, 1], F32, isOutput=False)
    bg2t = nc.declare_dram_parameter("bg2t", [P, ND], F32, isOutput=False)
    bb2t = nc.declare_dram_parameter("bb2t", [P, ND], F32, isOutput=False)
    out_ext = nc.declare_dram_parameter("out", [CL, Bi], F32, isOutput=True)

    with tile.TileContext(nc) as tc:
        with (
            tc.tile_pool(name="bigpool", bufs=1) as bigp,
            tc.tile_pool(name="smallpool", bufs=1) as smallp,
            tc.tile_pool(name="epool", bufs=2) as ep,
            tc.tile_pool(name="wspool", bufs=3) as wsp,
            tc.tile_pool(name="psum", bufs=2, space=bass.MemorySpace.PSUM) as pp,
            tc.tile_pool(name="psum_acc", bufs=1, space=bass.MemorySpace.PSUM) as ppa,
            tc.tile_pool(name="psum_s", bufs=2, space=bass.MemorySpace.PSUM) as pps,
        ):
            # ---------- load everything (small inputs first: the MLP and
            # per-dchunk stats gate the first exp, so caption/weight DMAs must
            # not queue behind the 9.4MB image) ----------
            cap_sb = smallp.tile([P, 2, D], F32)
            wm_sb = smallp.tile([P, 2, CL], F32)
            for ct in range(2):
                nc.sync.dma_start(cap_sb[:, ct, :], cap[ct * P:(ct + 1) * P, :])
                nc.sync.dma_start(wm_sb[:, ct, :], wm[ct * P:(ct + 1) * P, :])

            wg1_sb = smallp.tile([P, ND, H], F32)
            wb1_sb = smallp.tile([P, ND, H], F32)
            for dc in range(ND):
                nc.sync.dma_start(wg1_sb[:, dc, :], Wg1[dc * P:(dc + 1) * P, :])
                nc.sync.dma_start(wb1_sb[:, dc, :], Wb1[dc * P:(dc + 1) * P, :])
            wg2_sb = smallp.tile([P, D], F32)
            wb2_sb = smallp.tile([P, D], F32)
            nc.sync.dma_start(wg2_sb[:], Wg2[:, :])
            nc.sync.dma_start(wb2_sb[:], Wb2[:, :])
            bg1_sb = smallp.tile([P, 1], F32)
            bb1_sb = smallp.tile([P, 1], F32)
            nc.sync.dma_start(bg1_sb[:], bg1[:, :])
            nc.sync.dma_start(bb1_sb[:], bb1[:, :])
            bg2t_sb = smallp.tile([P, ND], F32)
            bb2t_sb = smallp.tile([P, ND], F32)
            nc.sync.dma_start(bg2t_sb[:], bg2t[:, :])
            nc.sync.dma_start(bb2t_sb[:], bb2t[:, :])

            x_all = bigp.tile([P, ND, BR], F32)
            for dc in range(ND):
                nc.sync.dma_start(x_all[:, dc, :], imgT[dc * P:(dc + 1) * P, :])

            ones_sb = smallp.tile([P, 1], F32)
            nc.vector.memset(ones_sb[:], 1.0)
            eps_bn_sb = smallp.tile([P, 1], F32)
            nc.vector.memset(eps_bn_sb[:], EPS_BN)
            eps_l2_sb = smallp.tile([P, 1], F32)
            nc.vector.memset(eps_l2_sb[:], EPS_L2)
            eps_rl2_sb = smallp.tile([P, 1], F32)
            nc.vector.memset(eps_rl2_sb[:], R * EPS_L2)

            # ---------- BN stats tiles (filled per-dchunk inside the main
            # loop so stats for chunk k+1 overlap compute on chunk k) ----------
            x_bf = bigp.tile([P, ND, BR], BF16)
            sumx = smallp.tile([P, ND], F32)
            sumsq = smallp.tile([P, ND], F32)
            mean = smallp.tile([P, ND], F32)
            var = smallp.tile([P, ND], F32)
            msq = smallp.tile([P, ND], F32)
            std = smallp.tile([P, ND], F32)
            rstd = smallp.tile([P, ND], F32)
            cbn = smallp.tile([P, ND], F32)
            grstd = smallp.tile([P, ND], F32)

            # ---------- cap_repr^T  [d, c] via mask matmul ----------
            crT = smallp.tile([P, ND, CL], F32)
            for dc in range(ND):
                pcr = pp.tile([P, CL], F32, tag="pcr")
                for ct in range(2):
                    nc.tensor.matmul(pcr[:], cap_sb[:, ct, dc * P:(dc + 1) * P],
                                     wm_sb[:, ct, :],
                                     start=(ct == 0), stop=(ct == 1))
                nc.vector.tensor_copy(crT[:, dc, :], pcr[:])

            # ---------- conditioning MLPs  gamma^T, beta^T [d, c] ----------
            gammaT = smallp.tile([P, ND, CL], F32)
            betaT = smallp.tile([P, ND, CL], F32)
            for (w1s, w2s, b1s, b2s, dstT) in (
                (wg1_sb, wg2_sb, bg1_sb, bg2t_sb, gammaT),
                (wb1_sb, wb2_sb, bb1_sb, bb2t_sb, betaT),
            ):
                ph = ppa.tile([P, CL], F32, tag="ph")
                for dc in range(ND):
                    nc.tensor.matmul(ph[:], w1s[:, dc, :], crT[:, dc, :],
                                     start=(dc == 0), stop=(dc == ND - 1))
                hT = smallp.tile([P, CL], F32, tag="hT")
                nc.vector.tensor_scalar(hT[:], ph[:], b1s[:], 0.0,
                                        op0=ALU.add, op1=ALU.max)
                for dc in range(ND):
                    pg = pp.tile([P, CL], F32, tag="pcr")
                    nc.tensor.matmul(pg[:], w2s[:, dc * P:(dc + 1) * P], hT[:],
                                     start=True, stop=True)
                    nc.vector.tensor_scalar_add(dstT[:, dc, :], pg[:],
                                                b2s[:, dc:dc + 1])

            # ---------- A, P1, P2 tiles (filled per-dchunk in main loop) ----
            A = smallp.tile([P, ND, CL], F32)
            P1 = smallp.tile([P, ND, CL], F32)
            P2 = smallp.tile([P, ND, CL], F32)

            # ---------- masked-column lhsT tiles ----------
            # lhs8[:, dc, c, :]: cols 0-3 hold cap_repr column c (others 0),
            # col 4+c holds ones. One matmul against rhs=[iv | iv2] then
            # accumulates s1 into PSUM row c and s2 into row 4+c with zero
            # contributions elsewhere (engine APs cannot start at a nonzero
            # partition, so per-row writes are not an option).
            lhs8 = smallp.tile([P, ND, CL, 8], F32)
            nc.vector.memset(lhs8[:], 0.0)
            for c in range(CL):
                for dc in range(ND):
                    nc.vector.tensor_copy(lhs8[:, dc, c, c:c + 1],
                                          crT[:, dc, c:c + 1])
                    nc.vector.memset(lhs8[:, dc, c, 4 + c:5 + c], 1.0)

            # ---------- main loop ----------
            G2 = 2 * Bi   # e and q reduced in one shared tree

            def tree_reduce(dst, src):
                """dst[P, 2*Bi] (fp32) = segmented sum over r of src[P, 2, Bi*R]
                (bf16, e in slab 0 / q in slab 1) via a binary tree of 2x-mode
                tensor_tensor adds; only the final 4-wide stage pays the 1x
                tensor_reduce rate."""
                s4 = src[:].rearrange("p a (b r) -> p a b r", r=R)
                t16 = wsp.tile([P, G2, 16], BF16, tag="t16")
                nc.vector.tensor_add(t16[:], s4[:, :, :, 0:16], s4[:, :, :, 16:32])
                t8 = wsp.tile([P, G2, 8], BF16, tag="t8")
                nc.vector.tensor_add(t8[:], t16[:, :, 0:8], t16[:, :, 8:16])
                t4 = wsp.tile([P, G2, 4], BF16, tag="t4")
                nc.vector.tensor_add(t4[:], t8[:, :, 0:4], t8[:, :, 4:8])
                t4b = wsp.tile([P, G2, 4], BF16, tag="t4b")
                nc.vector.tensor_add(
                    t4b[:], t4[:],
                    s4[:, :, :, 32:36].rearrange("p a b r -> p (a b) r"))
                t2 = wsp.tile([P, G2, 2], BF16, tag="t2")
                nc.vector.tensor_add(t2[:], t4b[:, :, 0:2], t4b[:, :, 2:4])
                nc.vector.tensor_add(
                    dst[:].rearrange("p (g o) -> p g o", o=1),
                    t2[:, :, 0:1], t2[:, :, 1:2])

            sims_sb = smallp.tile([CL, Bi], F32)
            ps12 = pps.tile([2 * CL, 2 * Bi], F32, tag="s12")
            for dc in range(ND):
                d1 = dc + 1
                # per-dchunk BN stats, fused into the bf16 copy + square
                # passes on the Scalar engine (accumulate port gives the sums)
                nc.scalar.activation(x_bf[:, dc, :], x_all[:, dc, :], AF.Copy,
                                     accum_out=sumx[:, dc:d1])
                junk = wsp.tile([P, BR], BF16, tag="junk")
                nc.scalar.activation(junk[:], x_all[:, dc, :], AF.Square,
                                     accum_out=sumsq[:, dc:d1])
                nc.vector.tensor_scalar_mul(mean[:, dc:d1], sumx[:, dc:d1],
                                            1.0 / BR)
                nc.vector.tensor_mul(msq[:, dc:d1], mean[:, dc:d1],
                                     mean[:, dc:d1])
                nc.vector.tensor_scalar_mul(var[:, dc:d1], sumsq[:, dc:d1],
                                            1.0 / BR)
                nc.vector.tensor_sub(var[:, dc:d1], var[:, dc:d1],
                                     msq[:, dc:d1])
                nc.scalar.activation(std[:, dc:d1], var[:, dc:d1], AF.Ln,
                                     bias=eps_bn_sb[:])
                nc.scalar.activation(rstd[:, dc:d1], std[:, dc:d1], AF.Exp,
                                     scale=-0.5)
                nc.vector.tensor_mul(cbn[:, dc:d1], mean[:, dc:d1],
                                     rstd[:, dc:d1])
                nc.vector.tensor_scalar_mul(cbn[:, dc:d1], cbn[:, dc:d1], -1.0)
                nc.vector.tensor_scalar_mul(grstd[:, dc:d1], rstd[:, dc:d1],
                                            GAMMA)
                nc.vector.tensor_scalar_mul(A[:, dc, :], gammaT[:, dc, :],
                                            grstd[:, dc:d1])
                nc.vector.tensor_scalar_mul(P1[:, dc, :], gammaT[:, dc, :],
                                            rstd[:, dc:d1])
                nc.vector.tensor_scalar_mul(P2[:, dc, :], gammaT[:, dc, :],
                                            cbn[:, dc:d1])
                nc.vector.tensor_add(P2[:, dc, :], P2[:, dc, :],
                                     betaT[:, dc, :])
                for c in range(CL):
                    eq = ep.tile([P, 2, BR], BF16, tag="eq")
                    nc.scalar.activation(eq[:, 0, :], x_all[:, dc, :], AF.Exp,
                                         scale=A[:, dc, c:c + 1])
                    nc.vector.tensor_mul(eq[:, 1, :], eq[:, 0, :],
                                         x_bf[:, dc, :])
                    sesq = wsp.tile([P, G2], F32, tag="sesq")
                    tree_reduce(sesq, eq)
                    rec = wsp.tile([P, Bi], F32, tag="rec")
                    nc.vector.reciprocal_approx_fast(rec[:], sesq[:, 0:Bi])
                    Sp = wsp.tile([P, Bi], F32, tag="Sp")
                    nc.vector.tensor_mul(Sp[:], sesq[:, Bi:G2], rec[:])
                    iviv2 = wsp.tile([P, 2, Bi], F32, tag="iviv2")
                    nc.vector.tensor_scalar(iviv2[:, 0, :], Sp[:],
                                            P1[:, dc, c:c + 1],
                                            P2[:, dc, c:c + 1],
                                            op0=ALU.mult, op1=ALU.add)
                    nc.vector.tensor_mul(iviv2[:, 1, :], iviv2[:, 0, :],
                                         iviv2[:, 0, :])
                    nc.tensor.matmul(
                        ps12[:], lhs8[:, dc, c, :],
                        iviv2[:].rearrange("p a b -> p (a b)"),
                        start=(dc == 0 and c == 0),
                        stop=(dc == ND - 1 and c == CL - 1))
            # ---------- s3[c] = sum_d cv^2 ----------
            ps3 = ppa.tile([CL, 1], F32, tag="s3")
            for c in range(CL):
                for dc in range(ND):
                    nc.tensor.matmul(ps3[:], lhs8[:, dc, c, 0:CL],
                                     crT[:, dc, c:c + 1],
                                     start=(c == 0 and dc == 0),
                                     stop=(c == CL - 1 and dc == ND - 1))

            # epilogue: sims[c, :] = s1 / ((sqrt(s2)+R*eps) * (sqrt(s3)+eps))
            # s1 sits in psum rows 0:4 cols 0:64; s2 in rows 4:8 cols 64:128.
            # Engine APs cannot start at partition 4, so a tiny SBUF-to-SBUF
            # DMA realigns s2 onto partitions 0:4.
            s1s2_sb = smallp.tile([2 * CL, 2 * Bi], F32)
            nc.vector.tensor_copy(s1s2_sb[:], ps12[:])
            s2_sb = smallp.tile([CL, Bi], F32)
            nc.sync.dma_start(s2_sb[:], s1s2_sb[CL:2 * CL, Bi:2 * Bi])
            # sq3 = sqrt(s3)+eps (Ln/Exp stay in the one loaded table set;
            # the ps3 dependency pins these after the main loop)
            sq3 = smallp.tile([CL, 1], F32)
            nc.scalar.activation(sq3[:], ps3[:], AF.Ln)
            nc.scalar.activation(sq3[:], sq3[:], AF.Exp, scale=0.5)
            nc.scalar.add(sq3[:], sq3[:], eps_l2_sb[:CL])
            den = wsp.tile([CL, Bi], F32, tag="den")
            nc.scalar.activation(den[:], s2_sb[:], AF.Ln)
            nc.scalar.activation(den[:], den[:], AF.Exp, scale=0.5)
            nc.scalar.add(den[:], den[:], eps_rl2_sb[:CL])
            nc.vector.tensor_scalar_mul(den[:], den[:], sq3[:])
            rden = wsp.tile([CL, Bi], F32, tag="rden")
            nc.vector.reciprocal(rden[:], den[:])
            nc.vector.tensor_mul(sims_sb[:], rden[:], s1s2_sb[0:CL, 0:Bi])

            nc.sync.dma_start(out_ext[:, :], sims_sb[:])

    nc.compile()
    return nc


def _prep_inputs(img_embed, cap_embed, Wg1, bg1, Wg2, bg2, Wb1, bb1, Wb2, bb2,
                 lens):
    """Host-side layout prep + per-core sharding. Returns in_maps (list of 8)."""
    f32 = np.float32
    imgT = np.ascontiguousarray(
        np.transpose(np.asarray(img_embed, f32), (2, 0, 1))).reshape(D, BR)
    capf = np.asarray(cap_embed, f32)
    lensf = np.asarray(lens)
    # ragged-mean weights: wfull[c, t] = (t < lens[c]) / lens[c]
    wfull = ((np.arange(T)[None, :] < lensf[:, None]) /
             lensf[:, None].astype(f32)).astype(f32)

    shared = {
        "imgT": imgT,
        "Wg1": np.ascontiguousarray(np.asarray(Wg1, f32)),
        "Wg2": np.ascontiguousarray(np.asarray(Wg2, f32)),
        "Wb1": np.ascontiguousarray(np.asarray(Wb1, f32)),
        "Wb2": np.ascontiguousarray(np.asarray(Wb2, f32)),
        "bg1": np.asarray(bg1, f32).reshape(H, 1),
        "bb1": np.asarray(bb1, f32).reshape(H, 1),
        "bg2t": np.ascontiguousarray(np.asarray(bg2, f32).reshape(ND, P).T),
        "bb2t": np.ascontiguousarray(np.asarray(bb2, f32).reshape(ND, P).T),
    }
    in_maps = []
    for i in range(NCORES):
        cs = slice(i * CL, (i + 1) * CL)
        cap_local = np.ascontiguousarray(capf[cs].reshape(CL * T, D))
        # block-diagonal mask-weight matrix [(c,t), c']
        wmat = np.zeros((CL * T, CL), f32)
        for cl in range(CL):
            wmat[cl * T:(cl + 1) * T, cl] = wfull[i * CL + cl]
        in_maps.append({**shared, "cap": cap_local, "wm": wmat})
    return in_maps


def kernel(**inputs) -> np.ndarray:
    global _COMPILED
    from concourse.bass_utils import run_bass_kernel_spmd

    if _COMPILED is None:
        _COMPILED = _build_graph()
    nc = _COMPILED

    in_maps = _prep_inputs(**inputs)
    res = run_bass_kernel_spmd(nc, in_maps, core_ids=list(range(NCORES)))
    sims = np.empty((Bi, Bc), np.float32)
    for i in range(NCORES):
        sims[:, i * CL:(i + 1) * CL] = res.results[i]["out"].T
    return sims


if __name__ == "__main__":
    # smoke test with random data
    rng = np.random.default_rng(0)
    ins = {
        "img_embed": rng.standard_normal((Bi, R, D), np.float32),
        "cap_embed": rng.standard_normal((Bc, T, D), np.float32),
        "Wg1": rng.standard_normal((D, H), np.float32) * 0.02,
        "bg1": np.zeros(H, np.float32),
        "Wg2": rng.standard_normal((H, D), np.float32) * 0.02,
        "bg2": np.zeros(D, np.float32),
        "Wb1": rng.standard_normal((D, H), np.float32) * 0.02,
        "bb1": np.zeros(H, np.float32),
        "Wb2": rng.standard_normal((H, D), np.float32) * 0.02,
        "bb2": np.zeros(D, np.float32),
        "lens": rng.integers(4, T - 4, Bc).astype(np.int32),
    }
    out = kernel(**ins)
    print(out.shape, out.dtype, np.abs(out).mean())
